# revision 21
# baseline (speedup 1.0000x reference)
"""BiLSTM-CRF Trainium2 kernel.

Strategy (8 NeuronCores, SPMD — one program, per-core data):
 - cores 0-3: forward LSTM over sequence quarters; cores 4-7: backward LSTM
   (fed the time-reversed sequence — the program is direction-agnostic).
 - Within a core the 2048 owned timesteps are split into C=128 chunk-columns
   of length L=16 processed simultaneously, turning the recurrent matvec
   into matmuls with C-column moving operands (weight streaming amortized,
   fp16 weights -> FWL fast weight load).
 - Exactness across chunk boundaries via a two-pass scheme: pass 1 runs each
   boundary's W=16-step lead-in from zero state (LSTM state contraction makes
   the boundary state converge); pass 2 runs the chunks from those states.
   fp16 xw / Whh / h with fp32 accumulation & cell state validated offline
   against the exact fp32 reference: identical Viterbi path, score to ~1e-6.
 - Gate groups are permuted to [i, f, o, g] so each PSUM bank holds one
   group; per-group add+activation overlaps the remaining groups' matmuls.
 - Phase 1 computes xw = x @ Wih.T + bias on-device into SBUF (fp16).
 - Phase 3 computes per-core partial logits h_dir @ W_out_half.T -> (2048,10).
 - Host combines partials into feats and runs Viterbi + backtrace.
"""

import numpy as np

SEQ, EMB, HDIR = 8192, 1024, 512
G = 4 * HDIR          # 2048 gate rows
TAGS, START, STOP = 10, 8, 9
NEG = -10000.0
NCORES = 8
CORE_T = SEQ // 4     # 2048 timesteps owned per core (per direction)
C = 128               # chunk-columns per core
L = CORE_T // C       # 16 chunk length
W = L                 # pass-1 lead-in steps (must be multiple of L)
ROWS = CORE_T + W     # xw rows per core (W lead-in + owned)
NB = 344              # phase-1 row-block (ROWS = 6*NB)

_built = {}


def _build_nc():
    import concourse.mybir as mybir
    import concourse.tile as tile
    from concourse import bacc
    from concourse.bass import ds

    f32 = mybir.dt.float32
    f16 = mybir.dt.float16
    AF = mybir.ActivationFunctionType
    OP = mybir.AluOpType

    nc = bacc.Bacc("TRN2", target_bir_lowering=False, debug=False,
                   num_devices=NCORES)

    xT = nc.dram_tensor("xT", [EMB, ROWS], f16, kind="ExternalInput")
    wihT = nc.dram_tensor("wihT", [EMB, G], f16, kind="ExternalInput")
    whhT = nc.dram_tensor("whhT", [HDIR, G], f16, kind="ExternalInput")
    woutT = nc.dram_tensor("woutT", [HDIR, TAGS], f16, kind="ExternalInput")
    biasPM = nc.dram_tensor("biasPM", [128, 16], f32, kind="ExternalInput")
    initmask = nc.dram_tensor("initmask", [128, 4 * C], f16, kind="ExternalInput")
    logits = nc.dram_tensor("logits", [CORE_T, TAGS], f32, kind="ExternalOutput")

    with tile.TileContext(nc) as tc:
        with (
            tc.tile_pool(name="persist", bufs=1) as persist,
            tc.tile_pool(name="work", bufs=3) as work,
        ):
            # xw resident in SBUF, fp16: (p, m, cc, t); (cc, t) = row cc*L+t
            xw = persist.tile([128, 16, 129, L], f16)

            # ---------------- Phase 1: xw = x @ Wih.T + bias ----------------
            with nc.named_scope("phase1"), \
                 tc.tile_pool(name="p1", bufs=1) as p1pool, \
                 tc.tile_pool(name="p1x", bufs=2) as p1x, \
                 tc.tile_pool(name="psum1", bufs=4, space="PSUM") as psum1:
                wih_sb = p1pool.tile([128, 8, G], f16)
                nc.sync.dma_start(
                    wih_sb[:], wihT[:].rearrange("(a p) g -> p a g", p=128))
                bias_sb = persist.tile([128, 16], f32)
                nc.sync.dma_start(bias_sb[:], biasPM[:])

                # contiguous row-blocks (chunk-aligned: 6x21 + 1x3 chunks);
                # psum rows are (cc, t)-major, scattered into xw's (t, cc)
                for rb in range(7):
                    cc0 = 21 * rb
                    ncc = 21 if rb < 6 else 3
                    nb = ncc * L
                    xt = p1x.tile([128, 8, 21 * L], f16, tag="xt")
                    nc.sync.dma_start(
                        xt[:, :, :nb],
                        xT[:, ds(cc0 * L, nb)].rearrange("(a p) r -> p a r",
                                                         p=128))
                    for m in range(16):
                        pm = psum1.tile([128, 21 * L], f32, tag="p1psum")
                        for k in range(8):
                            nc.tensor.matmul(
                                pm[:, :nb], wih_sb[:, k, ds(m * 128, 128)],
                                xt[:, k, :nb],
                                start=(k == 0), stop=(k == 7))
                        # add bias, downcast fp16; contiguous write
                        nc.vector.tensor_scalar_add(
                            xw[:, m, ds(cc0, ncc), :],
                            pm[:, :nb].rearrange("p (c t) -> p c t", t=L),
                            bias_sb[:, ds(m, 1)])

            # ---------------- Phase 2: chunked recurrence ----------------
            with tc.tile_pool(name="psum2", bufs=2, space="PSUM") as psum2:
                whh_sb = persist.tile([128, 4, G], f16)
                nc.sync.dma_start(
                    whh_sb[:], whhT[:].rearrange("(a p) g -> p a g", p=128))
                mask_sb = persist.tile([128, 4 * C], f16)
                nc.sync.dma_start(mask_sb[:], initmask[:])

                wout_sb = persist.tile([128, 4, TAGS], f16)
                nc.sync.dma_start(
                    wout_sb[:], woutT[:].rearrange("(a p) g -> p a g", p=128))
                lstage = persist.tile([128, L, TAGS], f32)
                h_st = persist.tile([128, 4 * C], f16)
                c_st = persist.tile([128, 4 * C], f32)
                nc.vector.memset(h_st[:], 0.0)
                nc.vector.memset(c_st[:], 0.0)

                # gate group layout after host-side permutation:
                # group 0 = i, 1 = f, 2 = o, 3 = g; group gr = m-tiles
                # 4gr..4gr+3 = psum bank gr = gates cols [4gr*C, (4gr+4)*C)
                def step(coff, t, store):
                    pgs = [psum2.tile([128, 4 * C], f32, tag=f"p2psum{gr}",
                                      name=f"pg{gr}")
                           for gr in range(4)]
                    gates = work.tile([128, 16 * C], f32, tag="gates")
                    ig = work.tile([128, 4 * C], f32, tag="ig")
                    tc_t = work.tile([128, 4 * C], f32, tag="tanh_c")

                    def gslice(gr):
                        return ds(4 * gr * C, 4 * C)

                    for gr in range(4):
                        pg = pgs[gr]
                        for m in range(4):
                            for k in range(4):
                                nc.tensor.matmul(
                                    pg[:, ds(m * C, C)],
                                    whh_sb[:, k, ds((4 * gr + m) * 128, 128)],
                                    h_st[:, ds(k * C, C)],
                                    start=(m == 0 and k == 0),
                                    stop=(k == 3),
                                    skip_group_check=True)
                        if gr == 3:
                            # o-group: add xw + sigmoid + h-update in halves
                            for hf in (0, 1):
                                s2 = ds(12 * C + hf * 2 * C, 2 * C)
                                sh = ds(hf * 2 * C, 2 * C)
                                nc.vector.scalar_tensor_tensor(
                                    gates[:, s2].rearrange(
                                        "p (m c) -> p m c", c=C),
                                    pg[:, ds(hf * 2 * C, 2 * C)].rearrange(
                                        "p (m c) -> p m c", c=C), 1.0,
                                    xw[:, ds(12 + 2 * hf, 2), ds(coff, C), t],
                                    op0=OP.mult, op1=OP.add)
                                nc.scalar.activation(gates[:, s2],
                                                     gates[:, s2], AF.Sigmoid)
                                nc.vector.tensor_mul(h_st[:, sh],
                                                     gates[:, s2],
                                                     tc_t[:, sh])
                        else:
                            nc.vector.scalar_tensor_tensor(
                                gates[:, gslice(gr)].rearrange(
                                    "p (m c) -> p m c", c=C),
                                pg[:].rearrange(
                                    "p (m c) -> p m c", c=C), 1.0,
                                xw[:, ds(4 * gr, 4), ds(coff, C), t],
                                op0=OP.mult, op1=OP.add)
                            nc.scalar.activation(
                                gates[:, gslice(gr)], gates[:, gslice(gr)],
                                AF.Tanh if gr == 0 else AF.Sigmoid)
                        if gr == 1:
                            # ig = i * g while f/o matmuls stream
                            nc.vector.tensor_mul(ig[:], gates[:, gslice(1)],
                                                 gates[:, gslice(0)])
                        if gr == 2:
                            # c = f*c + ig; tanh(c) while o matmuls stream
                            nc.vector.tensor_mul(c_st[:], gates[:, gslice(2)],
                                                 c_st[:])
                            nc.vector.tensor_add(c_st[:], c_st[:], ig[:])
                            nc.scalar.activation(tc_t[:], c_st[:], AF.Tanh)

                    # PE warm-keepers: tiny matmuls dependency-chained into
                    # the tail so the PE never idles a full HAM MID window.
                    nc.tensor.matmul(pgs[0][:1, :1], ig[:1, :1], ig[:1, :1],
                                     start=True, stop=True,
                                     skip_group_check=True)
                    nc.tensor.matmul(pgs[0][:1, 1:2], tc_t[:1, :1],
                                     tc_t[:1, :1], start=True, stop=True,
                                     skip_group_check=True)
                    nc.tensor.matmul(pgs[0][:1, 2:3], h_st[:1, :1],
                                     h_st[:1, :1], start=True, stop=True,
                                     skip_group_check=True)
                    if store is not None:
                        # partial logits for this step: (chunks, TAGS)
                        pl = psum2.tile([128, 4 * C], f32, tag="p2psum1",
                                        name="pl")
                        for k in range(4):
                            nc.tensor.matmul(pl[:, :TAGS],
                                             h_st[:, ds(k * C, C)],
                                             wout_sb[:, k, :],
                                             start=(k == 0), stop=(k == 3),
                                             skip_group_check=True)
                        nc.vector.tensor_copy(lstage[:, store, :],
                                              pl[:, :TAGS])
                    # pad PE busy through the tail with dependency-free
                    # weight loads so the HAM clock-gate never re-throttles
                    for j in range(14):
                        nc.tensor.ldweights(
                            whh_sb[:, j % 4, ds((j % 16) * 128, 128)])

                # pass 1: boundary lead-ins (W=L so all reads in block 0..C)
                with nc.named_scope("pass1"):
                    for t in range(W):
                        step(0, t, None)
                nc.vector.tensor_mul(h_st[:], h_st[:], mask_sb[:])
                nc.vector.tensor_mul(c_st[:], c_st[:], mask_sb[:])
                # pass 2: owned rows start at block 1
                with nc.named_scope("pass2"):
                    for t in range(L):
                        step(1, t, t)

                # logits: row c*L + t = lstage[c, t, :]
                with nc.named_scope("phase3"):
                    nc.sync.dma_start(
                        logits[:].rearrange("(c t) i -> c (t i)", t=L),
                        lstage[:].rearrange("p t i -> p (t i)"))

    nc.compile()
    return nc


# permutation of gate groups: [g, i, f, o] (PyTorch order is i, f, g, o)
_GPERM = [2, 0, 1, 3]


def _permute_gates(w):
    """w: (..., 2048) gate-major last axis -> permuted group order."""
    parts = [w[..., 512 * g:512 * (g + 1)] for g in _GPERM]
    return np.concatenate(parts, axis=-1)


def _in_maps(inputs):
    x = np.ascontiguousarray(inputs["batch"], np.float32)
    xb = x[::-1]
    maps = []
    mask = np.ones((128, 4 * C), np.float16)
    mask0 = mask.copy()
    mask0[:, 0::C] = 0.0  # zero chunk-0 init columns (c==0 for each j slice)
    for core in range(NCORES):
        d = core // 4          # 0 fwd, 1 bwd
        k = core % 4
        src = x if d == 0 else xb
        lo = k * CORE_T - W
        if lo < 0:
            xs = np.concatenate([np.zeros((W, EMB), np.float32),
                                 src[0:(k + 1) * CORE_T]], 0)
        else:
            xs = src[lo:(k + 1) * CORE_T]
        sfx = "_f" if d == 0 else "_b"
        bias = _permute_gates(
            (inputs["bih" + sfx] + inputs["bhh" + sfx]).astype(np.float32))
        wout_half = inputs["W_out"][:, :HDIR] if d == 0 else inputs["W_out"][:, HDIR:]
        maps.append({
            "xT": np.ascontiguousarray(xs.T, np.float16),
            "wihT": np.ascontiguousarray(
                _permute_gates(inputs["Wih" + sfx].T), np.float16),
            "whhT": np.ascontiguousarray(
                _permute_gates(inputs["Whh" + sfx].T), np.float16),
            "woutT": np.ascontiguousarray(wout_half.T, np.float16),
            "biasPM": np.ascontiguousarray(bias.reshape(16, 128).T, np.float32),
            "initmask": mask0 if k == 0 else mask,
        })
    return maps


def _viterbi_host(feats, trans):
    T = feats.shape[0]
    fv = np.full(TAGS, NEG, np.float32)
    fv[START] = 0.0
    bps = np.empty((T, TAGS), np.int32)
    for t in range(T):
        ntv = trans + fv[None, :]
        bps[t] = ntv.argmax(1)
        fv = (ntv.max(1) + feats[t]).astype(np.float32)
    terminal = fv + trans[STOP]
    best_last = int(terminal.argmax())
    score = np.float32(terminal[best_last])
    path = np.empty(T, np.int32)
    path[T - 1] = best_last
    cur = best_last
    for t in range(T - 1, 0, -1):
        cur = bps[t][cur]
        path[t - 1] = cur
    return score, path


def kernel(**inputs):
    from concourse.bass_utils import run_bass_kernel_spmd

    if "nc" not in _built:
        _built["nc"] = _build_nc()
    nc = _built["nc"]

    maps = _in_maps(inputs)
    res = run_bass_kernel_spmd(nc, maps, core_ids=list(range(NCORES)),
                               **_built.get("run_kwargs", {}))
    _built["last_res"] = res

    feats = np.zeros((SEQ, TAGS), np.float32)
    for core in range(NCORES):
        part = res.results[core]["logits"]
        k = core % 4
        if core < 4:
            feats[k * CORE_T:(k + 1) * CORE_T] += part
        else:
            feats[SEQ - (k + 1) * CORE_T: SEQ - k * CORE_T] += part[::-1]
    feats += inputs["b_out"].astype(np.float32)

    score, path = _viterbi_host(feats, np.asarray(inputs["transitions"], np.float32))
    return score, path


# revision 22
# speedup vs baseline: 1.0488x; 1.0488x over previous
"""BiLSTM-CRF Trainium2 kernel.

Strategy (8 NeuronCores, SPMD — one program, per-core data):
 - cores 0-3: forward LSTM over sequence quarters; cores 4-7: backward LSTM
   (fed the time-reversed sequence — the program is direction-agnostic).
 - Within a core the 2048 owned timesteps are split into C=128 chunk-columns
   of length L=16 processed simultaneously, turning the recurrent matvec
   into matmuls with C-column moving operands (weight streaming amortized,
   fp16 weights -> FWL fast weight load).
 - Exactness across chunk boundaries via a two-pass scheme: pass 1 runs each
   boundary's W=16-step lead-in from zero state (LSTM state contraction makes
   the boundary state converge); pass 2 runs the chunks from those states.
   fp16 xw / Whh / h with fp32 accumulation & cell state validated offline
   against the exact fp32 reference: identical Viterbi path, score to ~1e-6.
 - Gate groups are permuted to [i, f, o, g] so each PSUM bank holds one
   group; per-group add+activation overlaps the remaining groups' matmuls.
 - Phase 1 computes xw = x @ Wih.T + bias on-device into SBUF (fp16).
 - Phase 3 computes per-core partial logits h_dir @ W_out_half.T -> (2048,10).
 - Host combines partials into feats and runs Viterbi + backtrace.
"""

import numpy as np

SEQ, EMB, HDIR = 8192, 1024, 512
G = 4 * HDIR          # 2048 gate rows
TAGS, START, STOP = 10, 8, 9
NEG = -10000.0
NCORES = 8
CORE_T = SEQ // 4     # 2048 timesteps owned per core (per direction)
C = 128               # chunk-columns per core
L = CORE_T // C       # 16 chunk length
W = L                 # pass-1 lead-in steps (must be multiple of L)
ROWS = CORE_T + W     # xw rows per core (W lead-in + owned)
NB = 344              # phase-1 row-block (ROWS = 6*NB)

_built = {}


def _build_nc():
    import concourse.mybir as mybir
    import concourse.tile as tile
    from concourse import bacc
    from concourse.bass import ds

    f32 = mybir.dt.float32
    f16 = mybir.dt.float16
    AF = mybir.ActivationFunctionType
    OP = mybir.AluOpType

    nc = bacc.Bacc("TRN2", target_bir_lowering=False, debug=False,
                   num_devices=NCORES)

    xT = nc.dram_tensor("xT", [EMB, ROWS], f16, kind="ExternalInput")
    wihT = nc.dram_tensor("wihT", [EMB, G], f16, kind="ExternalInput")
    whhT = nc.dram_tensor("whhT", [HDIR, G], f16, kind="ExternalInput")
    woutT = nc.dram_tensor("woutT", [HDIR, TAGS], f16, kind="ExternalInput")
    biasPM = nc.dram_tensor("biasPM", [128, 16], f32, kind="ExternalInput")
    initmask = nc.dram_tensor("initmask", [128, 4 * C], f16, kind="ExternalInput")
    logits = nc.dram_tensor("logits", [CORE_T, TAGS], f32, kind="ExternalOutput")

    with tile.TileContext(nc) as tc:
        with (
            tc.tile_pool(name="persist", bufs=1) as persist,
            tc.tile_pool(name="work", bufs=3) as work,
        ):
            # xw resident in SBUF, fp16: (p, m, cc, t); (cc, t) = row cc*L+t
            xw = persist.tile([128, 16, 129, L], f16)

            # ---------------- Phase 1: xw = x @ Wih.T + bias ----------------
            with nc.named_scope("phase1"), \
                 tc.tile_pool(name="p1", bufs=1) as p1pool, \
                 tc.tile_pool(name="p1x", bufs=2) as p1x, \
                 tc.tile_pool(name="psum1", bufs=4, space="PSUM") as psum1:
                wih_sb = p1pool.tile([128, 8, G], f16)
                nc.sync.dma_start(
                    wih_sb[:], wihT[:].rearrange("(a p) g -> p a g", p=128))
                bias_sb = persist.tile([128, 16], f32)
                nc.sync.dma_start(bias_sb[:], biasPM[:])

                # contiguous row-blocks (chunk-aligned: 6x21 + 1x3 chunks);
                # psum rows are (cc, t)-major, scattered into xw's (t, cc)
                for rb in range(7):
                    cc0 = 21 * rb
                    ncc = 21 if rb < 6 else 3
                    nb = ncc * L
                    xt = p1x.tile([128, 8, 21 * L], f16, tag="xt")
                    nc.sync.dma_start(
                        xt[:, :, :nb],
                        xT[:, ds(cc0 * L, nb)].rearrange("(a p) r -> p a r",
                                                         p=128))
                    for m in range(16):
                        pm = psum1.tile([128, 21 * L], f32, tag="p1psum")
                        for k in range(8):
                            nc.tensor.matmul(
                                pm[:, :nb], wih_sb[:, k, ds(m * 128, 128)],
                                xt[:, k, :nb],
                                start=(k == 0), stop=(k == 7))
                        # add bias, downcast fp16; contiguous write
                        nc.vector.tensor_scalar_add(
                            xw[:, m, ds(cc0, ncc), :],
                            pm[:, :nb].rearrange("p (c t) -> p c t", t=L),
                            bias_sb[:, ds(m, 1)])

            # ---------------- Phase 2: chunked recurrence ----------------
            with tc.tile_pool(name="psum2", bufs=2, space="PSUM") as psum2:
                whh_sb = persist.tile([128, 4, G], f16)
                nc.sync.dma_start(
                    whh_sb[:], whhT[:].rearrange("(a p) g -> p a g", p=128))
                mask_sb = persist.tile([128, 4 * C], f16)
                nc.sync.dma_start(mask_sb[:], initmask[:])

                wout_sb = persist.tile([128, 4, TAGS], f16)
                nc.sync.dma_start(
                    wout_sb[:], woutT[:].rearrange("(a p) g -> p a g", p=128))
                lstage = persist.tile([128, L, TAGS], f32)
                h_st = persist.tile([128, 4 * C], f16)
                c_st = persist.tile([128, 4 * C], f32)
                nc.vector.memset(h_st[:], 0.0)
                nc.vector.memset(c_st[:], 0.0)

                # gate group layout after host-side permutation:
                # group 0 = i, 1 = f, 2 = o, 3 = g; group gr = m-tiles
                # 4gr..4gr+3 = psum bank gr = gates cols [4gr*C, (4gr+4)*C)
                def step(coff, t, store):
                    pgs = [psum2.tile([128, 4 * C], f32, tag=f"p2psum{gr}",
                                      name=f"pg{gr}")
                           for gr in range(4)]
                    gates = work.tile([128, 16 * C], f32, tag="gates")
                    ig = work.tile([128, 4 * C], f32, tag="ig")
                    tc_t = work.tile([128, 4 * C], f32, tag="tanh_c")

                    def gslice(gr):
                        return ds(4 * gr * C, 4 * C)

                    for gr in range(4):
                        pg = pgs[gr]
                        for m in range(4):
                            for k in range(4):
                                nc.tensor.matmul(
                                    pg[:, ds(m * C, C)],
                                    whh_sb[:, k, ds((4 * gr + m) * 128, 128)],
                                    h_st[:, ds(k * C, C)],
                                    start=(m == 0 and k == 0),
                                    stop=(k == 3),
                                    skip_group_check=True)
                        if gr == 3:
                            # o-group: add xw + sigmoid + h-update in halves
                            for hf in (0, 1):
                                s2 = ds(12 * C + hf * 2 * C, 2 * C)
                                sh = ds(hf * 2 * C, 2 * C)
                                nc.vector.scalar_tensor_tensor(
                                    gates[:, s2].rearrange(
                                        "p (m c) -> p m c", c=C),
                                    pg[:, ds(hf * 2 * C, 2 * C)].rearrange(
                                        "p (m c) -> p m c", c=C), 1.0,
                                    xw[:, ds(12 + 2 * hf, 2), ds(coff, C), t],
                                    op0=OP.mult, op1=OP.add)
                                nc.scalar.activation(gates[:, s2],
                                                     gates[:, s2], AF.Sigmoid)
                                nc.vector.tensor_mul(h_st[:, sh],
                                                     gates[:, s2],
                                                     tc_t[:, sh])
                        else:
                            nc.vector.scalar_tensor_tensor(
                                gates[:, gslice(gr)].rearrange(
                                    "p (m c) -> p m c", c=C),
                                pg[:].rearrange(
                                    "p (m c) -> p m c", c=C), 1.0,
                                xw[:, ds(4 * gr, 4), ds(coff, C), t],
                                op0=OP.mult, op1=OP.add)
                            nc.scalar.activation(
                                gates[:, gslice(gr)], gates[:, gslice(gr)],
                                AF.Tanh if gr == 0 else AF.Sigmoid)
                        if gr == 1:
                            # ig = i * g while f/o matmuls stream
                            nc.vector.tensor_mul(ig[:], gates[:, gslice(1)],
                                                 gates[:, gslice(0)])
                        if gr == 2:
                            # c = f*c + ig; tanh(c) while o matmuls stream
                            nc.vector.tensor_mul(c_st[:], gates[:, gslice(2)],
                                                 c_st[:])
                            nc.vector.tensor_add(c_st[:], c_st[:], ig[:])
                            nc.scalar.activation(tc_t[:], c_st[:], AF.Tanh)

                    # PE warm-keepers: tiny matmuls dependency-chained into
                    # the tail so the PE never idles a full HAM MID window.
                    nc.tensor.matmul(pgs[0][:1, :1], ig[:1, :1], ig[:1, :1],
                                     start=True, stop=True,
                                     skip_group_check=True)
                    nc.tensor.matmul(pgs[0][:1, 1:2], tc_t[:1, :1],
                                     tc_t[:1, :1], start=True, stop=True,
                                     skip_group_check=True)
                    nc.tensor.matmul(pgs[0][:1, 2:3], h_st[:1, :1],
                                     h_st[:1, :1], start=True, stop=True,
                                     skip_group_check=True)
                    if store is not None:
                        # partial logits for this step: (chunks, TAGS)
                        pl = psum2.tile([128, 4 * C], f32, tag="p2psum1",
                                        name="pl")
                        for k in range(4):
                            nc.tensor.matmul(pl[:, :TAGS],
                                             h_st[:, ds(k * C, C)],
                                             wout_sb[:, k, :],
                                             start=(k == 0), stop=(k == 3),
                                             skip_group_check=True)
                        nc.vector.tensor_copy(lstage[:, store, :],
                                              pl[:, :TAGS])

                # pass 1: boundary lead-ins (W=L so all reads in block 0..C)
                with nc.named_scope("pass1"):
                    for t in range(W):
                        step(0, t, None)
                nc.vector.tensor_mul(h_st[:], h_st[:], mask_sb[:])
                nc.vector.tensor_mul(c_st[:], c_st[:], mask_sb[:])
                # pass 2: owned rows start at block 1
                with nc.named_scope("pass2"):
                    for t in range(L):
                        step(1, t, t)

                # logits: row c*L + t = lstage[c, t, :]
                with nc.named_scope("phase3"):
                    nc.sync.dma_start(
                        logits[:].rearrange("(c t) i -> c (t i)", t=L),
                        lstage[:].rearrange("p t i -> p (t i)"))

    nc.compile()
    return nc


# permutation of gate groups: [g, i, f, o] (PyTorch order is i, f, g, o)
_GPERM = [2, 0, 1, 3]


def _permute_gates(w):
    """w: (..., 2048) gate-major last axis -> permuted group order."""
    parts = [w[..., 512 * g:512 * (g + 1)] for g in _GPERM]
    return np.concatenate(parts, axis=-1)


def _in_maps(inputs):
    x = np.ascontiguousarray(inputs["batch"], np.float32)
    xb = x[::-1]
    maps = []
    mask = np.ones((128, 4 * C), np.float16)
    mask0 = mask.copy()
    mask0[:, 0::C] = 0.0  # zero chunk-0 init columns (c==0 for each j slice)
    for core in range(NCORES):
        d = core // 4          # 0 fwd, 1 bwd
        k = core % 4
        src = x if d == 0 else xb
        lo = k * CORE_T - W
        if lo < 0:
            xs = np.concatenate([np.zeros((W, EMB), np.float32),
                                 src[0:(k + 1) * CORE_T]], 0)
        else:
            xs = src[lo:(k + 1) * CORE_T]
        sfx = "_f" if d == 0 else "_b"
        bias = _permute_gates(
            (inputs["bih" + sfx] + inputs["bhh" + sfx]).astype(np.float32))
        wout_half = inputs["W_out"][:, :HDIR] if d == 0 else inputs["W_out"][:, HDIR:]
        maps.append({
            "xT": np.ascontiguousarray(xs.T, np.float16),
            "wihT": np.ascontiguousarray(
                _permute_gates(inputs["Wih" + sfx].T), np.float16),
            "whhT": np.ascontiguousarray(
                _permute_gates(inputs["Whh" + sfx].T), np.float16),
            "woutT": np.ascontiguousarray(wout_half.T, np.float16),
            "biasPM": np.ascontiguousarray(bias.reshape(16, 128).T, np.float32),
            "initmask": mask0 if k == 0 else mask,
        })
    return maps


def _viterbi_host(feats, trans):
    T = feats.shape[0]
    fv = np.full(TAGS, NEG, np.float32)
    fv[START] = 0.0
    bps = np.empty((T, TAGS), np.int32)
    for t in range(T):
        ntv = trans + fv[None, :]
        bps[t] = ntv.argmax(1)
        fv = (ntv.max(1) + feats[t]).astype(np.float32)
    terminal = fv + trans[STOP]
    best_last = int(terminal.argmax())
    score = np.float32(terminal[best_last])
    path = np.empty(T, np.int32)
    path[T - 1] = best_last
    cur = best_last
    for t in range(T - 1, 0, -1):
        cur = bps[t][cur]
        path[t - 1] = cur
    return score, path


def kernel(**inputs):
    from concourse.bass_utils import run_bass_kernel_spmd

    if "nc" not in _built:
        _built["nc"] = _build_nc()
    nc = _built["nc"]

    maps = _in_maps(inputs)
    res = run_bass_kernel_spmd(nc, maps, core_ids=list(range(NCORES)),
                               **_built.get("run_kwargs", {}))
    _built["last_res"] = res

    feats = np.zeros((SEQ, TAGS), np.float32)
    for core in range(NCORES):
        part = res.results[core]["logits"]
        k = core % 4
        if core < 4:
            feats[k * CORE_T:(k + 1) * CORE_T] += part
        else:
            feats[SEQ - (k + 1) * CORE_T: SEQ - k * CORE_T] += part[::-1]
    feats += inputs["b_out"].astype(np.float32)

    score, path = _viterbi_host(feats, np.asarray(inputs["transitions"], np.float32))
    return score, path
